# revision 7
# baseline (speedup 1.0000x reference)
"""Multi-head self-attention (B=2, C=512, H=W=64, 8 heads) on 8 TRN2 cores.

Sharding: core i handles batch b = i//4 and pixel quarter q = i%4 (1024 of
4096 pixels) for ALL 8 heads.  K/V projections are computed over the full
pixel range (attention context), Q only over the local quarter.  No
collectives: each core's output slice [512, 1024] is fully local.

Per-core dataflow (all matmuls bf16 with fp32 PSUM accumulation):
  K,Q:  w-stationary  out[d, n] = sum_c wT[c, d] x[c, n].  The K bias is
        DROPPED: it shifts every logit of a query by a constant in the key
        index m, which softmax over m cancels.  Q keeps its bias.
  V:    x-stationary  out[m, d] = sum_c x[c, m] wvT[c, d], batched 4
        m-chunks per PSUM tile, one DVE copy each.  V bias is folded into
        the normalize step: (P(v+bv))/Z = PV/Z + bv.
  S^T:  out[m, n] = sum_d k[d, m] q[d, n], two heads packed via row
        tiling (K=64 each, tile_position (0,0)/(64,0)).
  exp:  split across three engines per [128,1024] unit: ACT runs true exp
        (scale=1/8); DVE/GPSIMD run a Schraudolph exp -- bits =
        round(s * 16*log2(e) + B) as int16, bitcast to bf16 (HW-verified:
        the f32->i16 cast rounds).  No max subtraction: |logits| <= ~7,
        and softmax is shift-invariant.
  AV:   TRANSPOSED: exp(S^T) is the STATIONARY operand ([128 m, 128 n])
        and the V chunk [128 m, 64] the moving one -- modeled cost 64 cols
        instead of 512 (the cost model prices output free size only).
        Accumulates out[n, d] over 32 m-chunks into [128, 512] PSUM per
        head; Z[n] = sum_m P[m, n] via a ones-column matmul into [128,16]
        PSUM.  Z lands in the PARTITION dim, so normalization is a plain
        per-partition tensor_scalar -- no broadcast machinery.
        PSUM accumulation streams sharing a 2KB zero-region use a single
        start=True on the region's first write (start zeroes the whole
        region; later streams accumulate from zero).
  norm: rz = reciprocal(Z); ao[n, d] = av*rz + bv (scalar_tensor_tensor).
  out-proj: needs channel-major layout, so each group's [128 n, 128 c]
        block is PE-transposed (cost 128/tile) into a bf16 PSUM tile and
        DVE-copied to SBUF; then the usual 4-chunk accumulation,
        out*gamma + xr with xr = x + gamma*b_out host-folded.
PSUM (8 banks): S 2x[128,1024]f32 = 4, AV+transpose pool 2x1 = 2, Z = 1,
misc (kq/V) 1; out-proj transients reuse the S slots at the end.
"""

import numpy as np
import ml_dtypes
from contextlib import ExitStack

import concourse.bass as bass
import concourse.mybir as mybir
import concourse.tile as tile
from concourse import bacc
from concourse import masks
from concourse.bass_utils import run_bass_kernel_spmd

F32 = mybir.dt.float32
BF16 = mybir.dt.bfloat16
I16 = mybir.dt.int16
AF = mybir.ActivationFunctionType
ALU = mybir.AluOpType

B, C, H, W = 2, 512, 64, 64
N = H * W              # 4096 pixels
NH, HD = 8, 64         # heads, head dim
NSL = N // 4           # pixels per core
NG = NH // 2           # head groups of 2
CK = C // 128          # c chunks
MT = N // 128          # m-chunks (key/value pixels)
KT = N // 512          # m-tiles of 512 for K proj
NT = NSL // 512        # n-tiles of 512 (query pixels)
NB = NSL // 128        # n-blocks of 128 (AV output partitions)
OT = C // 128          # output row chunks
VB = 4                 # V-proj m-chunks per batch
SCALE = HD ** -0.5

# Schraudolph bf16 exp: bits = round(s * A + B), bitcast int16->bf16.
EXP_A = 16.0 * float(np.log2(np.e))     # folds the 1/8 logit scale
EXP_B = 16249.0                          # 127*128 minus sawtooth correction

_cached = {}


def _build_kernel():
    nc = bacc.Bacc("TRN2", target_bir_lowering=False, debug=False,
                   num_devices=8)

    xb = nc.dram_tensor("xb", [C, N], BF16, kind="ExternalInput")
    xr = nc.dram_tensor("xr", [C, NSL], F32, kind="ExternalInput")
    wqkvT = nc.dram_tensor("wqkvT", [C, 2 * C], BF16, kind="ExternalInput")
    wvT = nc.dram_tensor("wvT", [C, C], BF16, kind="ExternalInput")
    bq = nc.dram_tensor("bq", [C, 1], F32, kind="ExternalInput")
    bvrow = nc.dram_tensor("bvrow", [1, C], BF16, kind="ExternalInput")
    woT = nc.dram_tensor("woT", [C, C], BF16, kind="ExternalInput")
    gam = nc.dram_tensor("gam", [1, 1], F32, kind="ExternalInput")
    out = nc.dram_tensor("out", [C, NSL], F32, kind="ExternalOutput")

    with tile.TileContext(nc) as tc:
        _emit_body(nc, tc, xb, xr, wqkvT, wvT, bq, bvrow, woT, gam, out)
    nc.compile()
    return nc


def _emit_body(nc, tc, xb, xr, wqkvT, wvT, bq, bvrow, woT, gam, out):
    with ExitStack() as ctx:
        ep = ctx.enter_context

        consts = ep(tc.tile_pool(name="consts", bufs=1))
        xpool = ep(tc.tile_pool(name="xpool", bufs=1))
        kqv = ep(tc.tile_pool(name="kqv", bufs=1))
        vpool = ep(tc.tile_pool(name="vpool", bufs=1))
        pexp = ep(tc.tile_pool(name="pexp", bufs=8))
        norm = ep(tc.tile_pool(name="norm", bufs=4))
        aop = ep(tc.tile_pool(name="aop", bufs=1))
        aon = ep(tc.tile_pool(name="aon", bufs=2))
        epi = ep(tc.tile_pool(name="epi", bufs=4))
        ps_s = ep(tc.tile_pool(name="ps_s", bufs=2, space="PSUM"))
        ps_av = ep(tc.tile_pool(name="ps_av", bufs=2, space="PSUM"))
        ps_z = ep(tc.tile_pool(name="ps_z", bufs=1, space="PSUM"))
        ps_m = ep(tc.tile_pool(name="ps_m", bufs=1, space="PSUM"))

        # ---- input loads -------------------------------------------------
        # xb arrives host-rotated so this core's query quarter is columns
        # 0:1024 (key order is permutation-invariant under softmax+AV).
        xbs = []
        wqk = []
        for c in range(CK):
            t = xpool.tile([128, N], BF16, tag=f"xb{c}", name=f"xbt{c}")
            nc.sync.dma_start(out=t[:, 0:512],
                              in_=xb[c * 128:(c + 1) * 128, 0:512])
            xbs.append(t)
            t = consts.tile([128, 2 * C], BF16, tag=f"wqk{c}", name=f"wqkt{c}")
            nc.sync.dma_start(out=t[:, 0:256],
                              in_=wqkvT[c * 128:(c + 1) * 128, 0:256])
            wqk.append(t)
        bqs = []
        for g in range(NG):
            t = consts.tile([128, 1], F32, tag=f"bq{g}", name=f"bqt{g}")
            nc.gpsimd.dma_start(out=t, in_=bq[g * 128:(g + 1) * 128, :])
            bqs.append(t)
        for c in range(CK):
            nc.sync.dma_start(out=xbs[c][:, 512:1024],
                              in_=xb[c * 128:(c + 1) * 128, 512:1024])
        for c in range(CK):
            nc.sync.dma_start(out=wqk[c][:, 256:2 * C],
                              in_=wqkvT[c * 128:(c + 1) * 128, 256:2 * C])
        for piece in range(2, KT):
            for c in range(CK):
                nc.sync.dma_start(
                    out=xbs[c][:, piece * 512:(piece + 1) * 512],
                    in_=xb[c * 128:(c + 1) * 128,
                           piece * 512:(piece + 1) * 512])
        wv = []
        for c in range(CK):
            t = consts.tile([128, C], BF16, tag=f"wv{c}", name=f"wvt{c}")
            nc.gpsimd.dma_start(out=t, in_=wvT[c * 128:(c + 1) * 128, :])
            wv.append(t)
        wo = []
        for c in range(CK):
            t = consts.tile([128, C], BF16, tag=f"wo{c}", name=f"wot{c}")
            nc.sync.dma_start(out=t, in_=woT[c * 128:(c + 1) * 128, :])
            wo.append(t)
        xrs = []
        for c in range(CK):
            t = xpool.tile([128, NSL], F32, tag=f"xr{c}", name=f"xrt{c}")
            nc.sync.dma_start(out=t, in_=xr[c * 128:(c + 1) * 128, :])
            xrs.append(t)
        gam128 = consts.tile([128, 1], F32, tag="gam")
        nc.gpsimd.dma_start(
            out=gam128,
            in_=bass.AP(tensor=gam.ap().tensor, offset=gam.ap().offset,
                        ap=[[0, 128], [1, 1]]),
        )
        bvb = consts.tile([128, C], BF16, tag="bvb")
        nc.gpsimd.dma_start(
            out=bvb,
            in_=bass.AP(tensor=bvrow.ap().tensor, offset=bvrow.ap().offset,
                        ap=[[0, 128], [1, C]]),
        )
        ident = consts.tile([128, 128], BF16, tag="ident")
        masks.make_identity(nc, ident[:])
        ones1 = consts.tile([128, 1], BF16, tag="ones1")
        nc.vector.memset(ones1, 1.0)

        # ---- persistent SBUF state --------------------------------------
        ksb = [kqv.tile([128, N], BF16, tag=f"k{g}", name=f"k{g}")
               for g in range(NG)]
        qsb = [kqv.tile([128, NSL], BF16, tag=f"q{g}", name=f"q{g}")
               for g in range(NG)]
        vsb = [[vpool.tile([128, VB * 128], BF16, tag=f"v{g}_{b}",
                           name=f"v{g}_{b}") for b in range(MT // VB)]
               for g in range(NG)]
        aoT = [aop.tile([128, NSL], BF16, tag=f"aoT{g}", name=f"aoT{g}")
               for g in range(NG)]

        def kq_piece(g, piece):
            p = ps_m.tile([128, 512], F32, tag="ps_m", name=f"kq{g}_{piece}")
            if piece < KT:
                mt = piece
                for c in range(CK):
                    nc.tensor.matmul(
                        p, wqk[c][:, g * 256 + 128:(g + 1) * 256],
                        xbs[c][:, mt * 512:(mt + 1) * 512],
                        start=(c == 0), stop=(c == CK - 1))
                nc.vector.tensor_copy(ksb[g][:, mt * 512:(mt + 1) * 512], p)
            else:
                ntt = piece - KT
                for c in range(CK):
                    nc.tensor.matmul(
                        p, wqk[c][:, g * 256:g * 256 + 128],
                        xbs[c][:, ntt * 512:(ntt + 1) * 512],
                        start=(c == 0), stop=(c == CK - 1))
                nc.vector.tensor_scalar_add(
                    qsb[g][:, ntt * 512:(ntt + 1) * 512], p, bqs[g])

        def v_batch(g, b):
            # V^T chunks 4b..4b+3 for group g: regions j of one PSUM bank.
            # Single start=True zeroes the whole 2KB region; streams j>0
            # accumulate from zero.
            p = ps_m.tile([128, 512], F32, tag="ps_m", name=f"vb{g}_{b}")
            for j in range(VB):
                mt = b * VB + j
                for c in range(CK):
                    nc.tensor.matmul(
                        p[:, j * 128:(j + 1) * 128],
                        xbs[c][:, mt * 128:(mt + 1) * 128],
                        wv[c][:, g * 128:(g + 1) * 128],
                        start=(c == 0 and j == 0), stop=(c == CK - 1),
                        skip_group_check=True)
            nc.scalar.copy(vsb[g][b][:, :], p)

        def out_proj(ntt):
            for ot in range(OT):
                p = ps_s.tile([128, 512], F32, tag="ps_s",
                              name=f"op{ot}_{ntt}")
                for g in range(NG):
                    nc.tensor.matmul(
                        p, wo[g][:, ot * 128:(ot + 1) * 128],
                        aoT[g][:, ntt * 512:(ntt + 1) * 512],
                        start=(g == 0), stop=(g == NG - 1))
                t2 = epi.tile([128, 512], F32, tag="t2")
                nc.vector.scalar_tensor_tensor(
                    out=t2, in0=p, scalar=gam128,
                    in1=xrs[ot][:, ntt * 512:(ntt + 1) * 512],
                    op0=ALU.mult, op1=ALU.add)
                nc.sync.dma_start(
                    out=out[ot * 128:(ot + 1) * 128,
                            ntt * 512:(ntt + 1) * 512],
                    in_=t2)

        # ---- preamble ----------------------------------------------------
        for piece in [0, KT, KT + 1]:
            kq_piece(0, piece)
        v_batch(0, 0)

        # ---- attention ---------------------------------------------------
        def s_exp(g, mt):
            # S^T for both heads of group g, chunk mt; exp engine rotates
            # per unit: par0 -> ACT (true exp), par1 -> Pool/Pool/DVE.
            pes = [None, None]
            for par in (1, 0):
                sp = ps_s.tile([128, NT * 512], F32, tag="ps_s")
                lo, hi = par * 64, par * 64 + 64
                for ntt in range(NT):
                    nc.tensor.matmul(
                        sp[:, ntt * 512:(ntt + 1) * 512],
                        ksb[g][lo:hi, mt * 128:(mt + 1) * 128],
                        qsb[g][lo:hi, ntt * 512:(ntt + 1) * 512],
                        start=True, stop=True,
                        tile_position=(par * 64, 0))
                if par == 0:
                    pe = pexp.tile([128, NT * 512], BF16, tag="pe")
                    nc.scalar.activation(pe, sp, AF.Exp, scale=SCALE)
                else:
                    pe = pexp.tile([128, NT * 512], I16, tag="pe")
                    nc.vector.tensor_scalar(pe, sp, EXP_A, EXP_B,
                                            ALU.mult, ALU.add)
                pes[par] = pe
            return pes

        for g in range(NG):
            avp = [ps_av.tile([128, NB * 64], F32, tag="ps_av",
                              name=f"av{g}_{par}") for par in range(2)]
            zp = ps_z.tile([128, 16], F32, tag="ps_z", name=f"z{g}")

            pes_ahead = s_exp(g, 0)
            for mt in range(MT):
                pes = pes_ahead
                if mt + 1 < MT:
                    pes_ahead = s_exp(g, mt + 1)
                if g == 0 and mt < KT - 1:
                    kq_piece(0, mt + 1)
                if g == 0 and mt % VB == 0 and mt // VB + 1 < MT // VB:
                    v_batch(0, mt // VB + 1)
                if g + 1 < NG:
                    if 16 <= mt < 16 + KT + NT:
                        kq_piece(g + 1, mt - 16)
                    if mt % VB == 2:
                        v_batch(g + 1, mt // VB)
                b, j = mt // VB, mt % VB
                for par in range(2):
                    stat = pes[par]
                    for nb in range(NB):
                        st = stat[:, nb * 128:(nb + 1) * 128]
                        if stat.dtype == I16:
                            st = st.bitcast(BF16)
                        nc.tensor.matmul(
                            avp[par][:, nb * 64:(nb + 1) * 64],
                            st, vsb[g][b][:, j * 128 + par * 64:j * 128 + (par + 1) * 64],
                            start=(mt == 0 and nb == 0),
                            stop=(mt == MT - 1),
                            skip_group_check=True)
                        nc.tensor.matmul(
                            zp[:, par * 8 + nb:par * 8 + nb + 1],
                            st, ones1,
                            start=(mt == 0 and par == 0 and nb == 0),
                            stop=(mt == MT - 1),
                            skip_group_check=True)
            # ---- normalize + transpose ----------------------------------
            aosn = [aon.tile([128, 128], BF16, tag=f"ao{nb}",
                             name=f"ao{g}_{nb}") for nb in range(NB)]
            for par in range(2):
                h = 2 * g + par
                rz = norm.tile([128, 8], F32, tag="rz")
                nc.vector.reciprocal(rz, zp[:, par * 8:(par + 1) * 8])
                for nb in range(NB):
                    nc.vector.scalar_tensor_tensor(
                        out=aosn[nb][:, par * 64:(par + 1) * 64],
                        in0=avp[par][:, nb * 64:(nb + 1) * 64],
                        scalar=rz[:, nb:nb + 1],
                        in1=bvb[:, h * 64:(h + 1) * 64],
                        op0=ALU.mult, op1=ALU.add)
            tp = ps_av.tile([128, NSL], BF16, tag="ps_av", name=f"tp{g}")
            for nb in range(NB):
                nc.tensor.matmul(
                    tp[:, nb * 128:(nb + 1) * 128], aosn[nb][:, :], ident,
                    is_transpose=True, start=(nb == 0), stop=(nb == NB - 1),
                    skip_group_check=True)
            nc.vector.tensor_copy(aoT[g][:, :], tp)

        # ---- output projection ------------------------------------------
        for ntt in range(NT):
            out_proj(ntt)


def _prep_in_maps(x, w_qkv, b_qkv, w_out, b_out, gamma):
    bf = ml_dtypes.bfloat16
    x = np.asarray(x, np.float32).reshape(B, C, N)
    w_qkv = np.asarray(w_qkv, np.float32)
    b_qkv = np.asarray(b_qkv, np.float32)
    w_out = np.asarray(w_out, np.float32)
    b_out = np.asarray(b_out, np.float32)
    gamma = np.asarray(gamma, np.float32)

    wqkT = w_qkv[:2 * C].T                                         # [C, 2C]
    wqkvT = np.empty((C, 2 * C), np.float32)
    for g in range(NG):
        wqkvT[:, g * 256:g * 256 + 128] = wqkT[:, g * 128:(g + 1) * 128]
        wqkvT[:, g * 256 + 128:(g + 1) * 256] = \
            wqkT[:, C + g * 128:C + (g + 1) * 128]
    wqkvT = wqkvT.astype(bf)
    wvT = np.ascontiguousarray(w_qkv[2 * C:].T).astype(bf)         # [C, C]
    bvrow = b_qkv[2 * C:].reshape(1, C).astype(bf)
    woT = np.ascontiguousarray(w_out.T).astype(bf)
    bq = b_qkv[:C].reshape(C, 1)
    gam = gamma.reshape(1, 1)

    xbf = x.astype(bf)
    in_maps = []
    for i in range(8):
        b, q = i // 4, i % 4
        sl = slice(q * NSL, (q + 1) * NSL)
        in_maps.append({
            "xb": np.roll(xbf[b], -q * NSL, axis=1),
            "xr": np.ascontiguousarray(x[b][:, sl])
                  + (gamma.reshape(()) * b_out)[:, None].astype(np.float32),
            "wqkvT": wqkvT, "wvT": wvT, "bq": bq, "bvrow": bvrow,
            "woT": woT, "gam": gam,
        })
    return in_maps


def _assemble(results):
    full = np.empty((B, C, N), np.float32)
    for i in range(8):
        b, q = i // 4, i % 4
        full[b][:, q * NSL:(q + 1) * NSL] = results[i]["out"]
    return full.reshape(B, C, H, W)


def kernel(x, w_qkv, b_qkv, w_out, b_out, gamma):
    if "nc" not in _cached:
        _cached["nc"] = _build_kernel()
    nc = _cached["nc"]
    in_maps = _prep_in_maps(x, w_qkv, b_qkv, w_out, b_out, gamma)
    res = run_bass_kernel_spmd(nc, in_maps, core_ids=list(range(8)))
    return _assemble(res.results)
